# revision 5
# baseline (speedup 1.0000x reference)
"""Haar DWT decoder (2-level inverse, zero details) as a Trainium2 Bass kernel.

out[b, c, j, k] = z[b].reshape(C, 128, 128)[c, j//4, k//4] * 0.25
i.e. a 4x4 nearest-neighbor upsample scaled by 1/4.

Data-parallel over batch: 128 samples -> 16 per core on 8 NeuronCores.

Per-core shape of the problem: read 3 MiB of z, write 48 MiB of output
through 16 SDMA engines at ~26.5 GB/s each (~424 GB/s aggregate), so the
floor is ~122 us of streaming plus the pipeline lead-in.

All DMA (loads and stores) goes through the two HWDGE rings (sync +
scalar). SWDGE (gpsimd) is deliberately unused: its descriptor rings
live on SBUF partitions whose AXI ports are shared with SDMA engines
7/15, and descriptor fetches from those rings are the known cause of
the "engines 7/15 run ~18% slow" straggler mode that adds ~20 us of
tail. HWDGE has no SBUF descriptor ring (and ~0.6 us vs ~1 us
first-byte latency). Loads ride the store rings' FIFO: six are issued
up front, then one after each store — a load is only ~0.5 us of ring
time, and compute stays ~20 us ahead of the ring cadence, so they never
stall the stores.
"""

import numpy as np

import concourse.bass as bass
import concourse.mybir as mybir
import concourse.tile as tile
from concourse.bass_utils import run_bass_kernel_spmd

# The walrus build in this container rejects instructions carrying more than
# one sync-wait command (codegen: "Too many sync wait commands" — observed on
# a Drain with 3 waits and a DMACopy with 2). Tile freely attaches several
# waits to one instruction, so after tracing we split the excess onto NOPs
# inserted just before the instruction on the same engine; sequential
# dispatch on one engine makes that equivalent.
_MAX_WAITS = 1


def _split_excess_waits(nc: bass.Bass) -> None:
    for f in nc.m.functions:
        for bb in f.blocks:
            insns = bb.instructions
            # Iterate over a snapshot; mutate the live list via insert.
            for ins in list(insns):
                si = ins.sync_info
                if si is None or not si.on_wait or len(si.on_wait) <= _MAX_WAITS:
                    continue
                waits = list(si.on_wait)
                keep = waits[-_MAX_WAITS:]
                spill = waits[:-_MAX_WAITS]
                pos = insns.index(ins)
                nops = []
                for i in range(0, len(spill), _MAX_WAITS):
                    nop = nc.engines[ins.engine].nop(nofuse=True).ins
                    # nop() appended itself to the current bb; pull it out.
                    cur = nc.cur_bb.bb.instructions
                    assert cur[-1] is nop
                    cur.pop()
                    nop.sync_info = mybir.SyncInfo(
                        on_wait=spill[i : i + _MAX_WAITS], on_update=[]
                    )
                    nops.append(nop)
                insns[pos:pos] = nops
                ins.sync_info = mybir.SyncInfo(
                    on_wait=keep, on_update=list(si.on_update)
                )

# Problem constants (hardcoded: module config out_shape=(3,512,512), levels=2)
BATCH = 128
C = 3
CAH = 128  # coarse-approximation spatial dims
CAW = 128
S = 4      # 2**levels upsample factor
H = 512
W = 512
N_CORES = 8
B_SHARD = BATCH // N_CORES  # 16

# Loads issued ahead of the compute loop (ring FIFO keeps them ~6 samples
# ahead of the stores, which is far more than compute needs).
PRELOAD = 6

F32 = mybir.dt.float32


def _build_nc(b_shard: int = B_SHARD) -> bass.Bass:
    nc = bass.Bass("TRN2", target_bir_lowering=False, debug=False)
    z = nc.dram_tensor("z", [b_shard, C * CAH * CAW], F32, kind="ExternalInput").ap()
    out = nc.dram_tensor("out", [b_shard, C, H, W], F32, kind="ExternalOutput").ap()

    def ring(i: int):
        return nc.sync if i % 2 == 0 else nc.scalar

    with tile.TileContext(nc) as tc:
        with (
            tc.tile_pool(name="zin", bufs=PRELOAD + 1) as zin_pool,
            tc.tile_pool(name="wide", bufs=6) as w_pool,
        ):
            zts: list = []

            def issue_load(b: int) -> None:
                # Load z[b] as [jc=128 partitions, (c, kc) free] on the HWDGE
                # ring matching the sample's store parity.
                zt = zin_pool.tile([CAH, C * CAW], F32)
                zts.append(zt)
                src = z[b].rearrange("(c jc kc) -> jc c kc", c=C, jc=CAH, kc=CAW)
                ring(b).dma_start(
                    out=zt[:].rearrange("p (c kc) -> p c kc", c=C), in_=src
                )

            for b in range(min(PRELOAD, b_shard)):
                issue_load(b)

            for b in range(b_shard):
                zt = zts[b]
                zv = zt[:].rearrange("p (c kc) -> p c kc", c=C)

                # Materialize the upsampled sample in SBUF: partition jc holds
                # output rows 4*jc..4*jc+3 of every channel, free layout
                # (c, jr, k), so output DMAs are fully contiguous with 8 KiB
                # descriptor runs.
                w2 = w_pool.tile([CAH, C * S * W], F32, tag="wide")
                w2v = w2[:].rearrange(
                    "p (c jr kc kr) -> p c jr kc kr", c=C, jr=S, kc=CAW, kr=S
                )
                w2f = w2[:].rearrange("p (c jr k) -> p c jr k", c=C, jr=S)

                # Width-expand x4 (with the 1/4 scale) into the jr=0 rows in a
                # single contiguous-write op via a 0-stride (broadcast) input;
                # height-replicate into jr=1..3 split across DVE and ACT.
                # (gpsimd's tensor_copy runs ~4x slower than ACT — don't use
                # it.)
                zb = zv.unsqueeze(3).broadcast_to([CAH, C, CAW, S])
                nc.vector.tensor_scalar_mul(w2v[:, :, 0, :, :], zb, 0.25)
                nc.scalar.copy(w2f[:, :, 1, :], w2f[:, :, 0, :])
                nc.vector.tensor_copy(w2f[:, :, 2, :], w2f[:, :, 0, :])
                if b < 2:
                    # First sample per ring: balance the jr=3 copy across ACT
                    # (1 ch) and DVE (2 ch) so the two engines finish together
                    # (~2.0 us after the mul instead of 3.1 us serialized on
                    # ACT) — pulls the ring's first store earlier. Not worth
                    # the extra instructions in steady state, where compute
                    # has ~2x slack over the store cadence.
                    nc.scalar.copy(w2f[:, 0, 3, :], w2f[:, 0, 0, :])
                    nc.vector.tensor_copy(w2f[:, 1:3, 3, :], w2f[:, 1:3, 0, :])
                else:
                    nc.scalar.copy(w2f[:, :, 3, :], w2f[:, :, 0, :])

                # One fully-contiguous 3 MiB DMA per sample; alternate between
                # the two HWDGE rings for descriptor-gen overlap. (Splitting
                # any DMAs per-channel measurably depresses the SDMA rate —
                # keep them whole.)
                ov = out[b].rearrange("c (jc jr) k -> jc c (jr k)", jr=S)
                ring(b).dma_start(
                    out=ov, in_=w2[:].rearrange("p (c jrk) -> p c jrk", c=C)
                )

                # The next load goes on the same ring right after this store:
                # it drains ~6 samples before compute needs it.
                if b + PRELOAD < b_shard:
                    issue_load(b + PRELOAD)

    _split_excess_waits(nc)
    return nc


_NC_CACHE: dict[int, bass.Bass] = {}


def _get_nc(b_shard: int = B_SHARD) -> bass.Bass:
    if b_shard not in _NC_CACHE:
        _NC_CACHE[b_shard] = _build_nc(b_shard)
    return _NC_CACHE[b_shard]


def kernel(z: np.ndarray) -> np.ndarray:
    z = np.ascontiguousarray(z, dtype=np.float32)
    assert z.shape == (BATCH, C * CAH * CAW), z.shape
    nc = _get_nc()
    in_maps = [
        {"z": z[i * B_SHARD : (i + 1) * B_SHARD]} for i in range(N_CORES)
    ]
    res = run_bass_kernel_spmd(nc, in_maps, list(range(N_CORES)))
    return np.concatenate([res.results[i]["out"] for i in range(N_CORES)], axis=0)
